# revision 19
# baseline (speedup 1.0000x reference)
"""GCN (4-layer + 3 skip convs) on 8 Trainium2 NeuronCores.

Math: gcn_conv(h, W, b) = A @ (h @ W) + b = (A @ h) @ W + b, where A is the
(dst,src)-adjacency with multiplicity.  Since segment_sum is linear we
aggregate RAW features once per layer (agg_i = A @ h_i) and apply both the
main-conv weight and the skip-conv weight to the same aggregate:
    h_{i+1} = relu(agg_i @ gcn_W[i] + gcn_b[i])          i = 0,1,2
    skip_j  = relu(agg_{j+1} @ skip_W[j] + skip_b[j])    j = 0,1,2
h4 (gcn layer 3) is never used by the outputs, so only 4 aggregation passes.

Distribution: nodes are dst-sharded 6400/core (N padded 50000->51200).  Each
core gathers source features for its own edges with hardware dma_gather
(HBM->SBUF), builds one-hot selection tiles S on VectorE, and aggregates with
TensorE matmuls into PSUM:  aggT[feat,dst] += msgs[e,feat]^T-contract S[e,dst].
After each layer the per-core h-shard is AllGathered (split in two halves so
the int16 gather indices stay < 32768 and the collective pipelines).
"""

import os
import sys
import hashlib

import numpy as np

sys.path.insert(0, "/opt/trn_rl_repo")

N_RAW = 50000
D = 128
NCORES = 8
SHARD = 6400          # dst nodes per core (padded N = 51200)
HALF = SHARD // 2     # 3200; h is stored as two tensors of 8*HALF rows
NP_ = NCORES * SHARD  # 51200
NW = SHARD // 128     # 50 windows of 128 dst nodes per core
WIN = 128
STAGE_W = 3           # windows per gather stage
PAD_DST = 255.0       # sentinel dst_local for padding edges (never matches iota)

_cache = {}


def _prep_edges(edge_index):
    """Host-side: build per-core gather-index / dst-value streams with a
    schedule (chunks per window/half) shared across all 8 cores (SPMD)."""
    src = edge_index[0].astype(np.int64)
    dst = edge_index[1].astype(np.int64)
    E = src.shape[0]

    core = dst // SHARD
    w = (dst % SHARD) // WIN
    dloc = (dst % WIN).astype(np.float16)
    # h-storage row layout (matches the SBUF->DRAM AG staging DMA, which is a
    # plain [128, HALF] partition-major copy of sb_h):
    #   node n -> rank = n//SHARD, posh = (n%SHARD)%HALF, w = posh//128,
    #             p = posh%128, row = rank*HALF + p*(HALF//128) + w
    pos = src % SHARD
    is_hi = pos >= HALF
    posh = pos - is_hi * HALF
    grow = (src // SHARD) * HALF + (posh % WIN) * (HALF // WIN) + posh // WIN

    gid = (core * NW + w) * 2 + is_hi.astype(np.int64)
    NG = NCORES * NW * 2
    counts_flat = np.bincount(gid, minlength=NG)
    counts = counts_flat.reshape(NCORES, NW, 2)
    maxc = counts.max(axis=0)                       # [NW, 2]
    nch = np.maximum((maxc + WIN - 1) // WIN, 1)    # chunks per window/half
    nch_lo = nch[:, 0].astype(np.int64)
    nch_hi = nch[:, 1].astype(np.int64)
    lo_off = np.concatenate([[0], np.cumsum(nch_lo)])   # chunk offsets, LO stream
    hi_off = np.concatenate([[0], np.cumsum(nch_hi)])
    C_LO = int(lo_off[-1])
    C_HI = int(hi_off[-1])
    # concatenated S stream: per window [lo chunks][hi chunks]
    cat_off = np.concatenate([[0], np.cumsum(nch_lo + nch_hi)])
    C_TOT = int(cat_off[-1])

    order = np.argsort(gid, kind="stable")
    gsort = gid[order]
    starts = np.concatenate([[0], np.cumsum(counts_flat)])
    ranks = np.arange(E, dtype=np.int64) - starts[gsort]
    w_of = (gsort // 2) % NW
    hi_of = (gsort % 2).astype(bool)
    core_of = gsort // (NW * 2)

    # slot in per-core LO/HI gather streams
    slot_gath = np.where(hi_of, hi_off[w_of], lo_off[w_of]) * WIN + ranks
    # slot in per-core concatenated dst-value stream
    slot_cat = np.where(hi_of, (cat_off[w_of] + nch_lo[w_of]), cat_off[w_of]) * WIN + ranks

    rows = grow[order]
    dl = dloc[order]

    idx_lo = np.zeros((NCORES, C_LO * WIN), np.int16)
    idx_hi = np.zeros((NCORES, C_HI * WIN), np.int16)
    dval = np.full((NCORES, C_TOT * WIN), PAD_DST, np.float16)

    m = ~hi_of
    idx_lo[core_of[m], slot_gath[m]] = rows[m].astype(np.int16)
    m = hi_of
    idx_hi[core_of[m], slot_gath[m]] = rows[m].astype(np.int16)
    dval[core_of, slot_cat] = dl

    def wrap16(a):   # [n] int16 -> [128, n//16] wrapped per 16, replicated x8
        n = a.shape[0]
        t = a.reshape(n // 16, 16).T                # [16, n//16]
        return np.tile(t, (8, 1)).copy()

    def chunkmaj(a):  # [C*128] -> [128, C]
        return np.ascontiguousarray(a.reshape(-1, WIN).T)

    idx_lo_w = np.stack([wrap16(idx_lo[r]) for r in range(NCORES)])
    idx_hi_w = np.stack([wrap16(idx_hi[r]) for r in range(NCORES)])
    dval_c = np.stack([chunkmaj(dval[r]) for r in range(NCORES)])

    sched = dict(
        nch_lo=[int(v) for v in nch_lo], nch_hi=[int(v) for v in nch_hi],
        lo_off=[int(v) for v in lo_off], hi_off=[int(v) for v in hi_off],
        cat_off=[int(v) for v in cat_off],
        C_LO=C_LO, C_HI=C_HI, C_TOT=C_TOT,
    )
    return sched, idx_lo_w, idx_hi_w, dval_c


def _build_program(sched, n_passes=4):
    import concourse.bacc as bacc
    import concourse.bass as bass
    import concourse.tile as tile
    from concourse import mybir
    from contextlib import ExitStack

    f16 = mybir.dt.float16
    f32 = mybir.dt.float32
    i16 = mybir.dt.int16
    AF = mybir.ActivationFunctionType
    OP = mybir.AluOpType

    NCH_LO = sched["nch_lo"]
    NCH_HI = sched["nch_hi"]
    LO_OFF = sched["lo_off"]
    HI_OFF = sched["hi_off"]
    CAT_OFF = sched["cat_off"]
    C_LO = sched["C_LO"]
    C_HI = sched["C_HI"]
    C_TOT = sched["C_TOT"]

    n_stages = (NW + STAGE_W - 1) // STAGE_W
    max_st_lo = max(LO_OFF[min(s * STAGE_W + STAGE_W, NW)] - LO_OFF[s * STAGE_W]
                    for s in range(n_stages))
    max_st_hi = max(HI_OFF[min(s * STAGE_W + STAGE_W, NW)] - HI_OFF[s * STAGE_W]
                    for s in range(n_stages))
    max_win_ct = max(NCH_LO[w] + NCH_HI[w] for w in range(NW))

    nc = bacc.Bacc("TRN2", target_bir_lowering=False, debug=False)

    p_xT = nc.declare_dram_parameter("xT", [128, SHARD], f16, isOutput=False)
    p_ilo = nc.declare_dram_parameter("idx_lo", [128, C_LO * 8], i16, isOutput=False)
    p_ihi = nc.declare_dram_parameter("idx_hi", [128, C_HI * 8], i16, isOutput=False)
    p_dval = nc.declare_dram_parameter("dval", [128, C_TOT], f16, isOutput=False)
    p_w = nc.declare_dram_parameter("wcat", [128, 7 * 128], f16, isOutput=False)
    p_bias = nc.declare_dram_parameter("bias_bc", [128, 4 * 128], f32, isOutput=False)
    p_bcol = nc.declare_dram_parameter("bias_col", [128, 4], f32, isOutput=False)
    p_rw = nc.declare_dram_parameter("rw", [128, 4], f32, isOutput=False)
    p_iota = nc.declare_dram_parameter("iota", [128, 128], f16, isOutput=False)
    o_feat = nc.declare_dram_parameter("out_featT", [3 * 128, SHARD], f32, isOutput=True)
    o_log = nc.declare_dram_parameter("logits", [128, NW], f32, isOutput=True)
    o_sig = nc.declare_dram_parameter("sig", [128, NW], f32, isOutput=True)

    hA = [nc.dram_tensor(f"hA{i}", [NCORES * HALF, 128], f16, addr_space="Shared")
          for i in range(4)]
    hB = [nc.dram_tensor(f"hB{i}", [NCORES * HALF, 128], f16, addr_space="Shared")
          for i in range(4)]
    agA = [nc.dram_tensor(f"agA{i}", [128, HALF], f16) for i in range(4)]
    agB = [nc.dram_tensor(f"agB{i}", [128, HALF], f16) for i in range(4)]

    groups = [list(range(NCORES))]

    with ExitStack() as ctx:
        tc = ctx.enter_context(tile.TileContext(nc))
        consts = ctx.enter_context(tc.tile_pool(name="consts", bufs=1))
        xpool = ctx.enter_context(tc.tile_pool(name="xp", bufs=3))
        mlo_pool = ctx.enter_context(tc.tile_pool(name="mlo", bufs=2))
        mhi_pool = ctx.enter_context(tc.tile_pool(name="mhi", bufs=2))
        s_pool = ctx.enter_context(tc.tile_pool(name="sp", bufs=2))
        agg_sb = ctx.enter_context(tc.tile_pool(name="aggsb", bufs=3))
        agg_ps = ctx.enter_context(tc.tile_pool(name="aggps", bufs=2, space="PSUM"))
        head_ps = ctx.enter_context(tc.tile_pool(name="headps", bufs=2, space="PSUM"))
        log_ps = ctx.enter_context(tc.tile_pool(name="logps", bufs=1, space="PSUM"))

        sb_ilo = consts.tile([128, C_LO * 8], i16, name="sb_ilo")
        sb_ihi = consts.tile([128, C_HI * 8], i16, name="sb_ihi")
        sb_dval = consts.tile([128, C_TOT], f16, name="sb_dval")
        sb_w = consts.tile([128, 7 * 128], f16, name="sb_w")
        sb_bias = consts.tile([128, 4 * 128], f32, name="sb_bias")
        sb_bcol = consts.tile([128, 4], f32, name="sb_bcol")
        sb_rw = consts.tile([128, 4], f32, name="sb_rw")
        sb_iota = consts.tile([128, 128], f16, name="sb_iota")
        sb_h = consts.tile([128, NW * 128], f16, name="sb_h")
        sb_skipT = consts.tile([128, NW * 128], f32, name="sb_skipT")
        sb_log = consts.tile([128, NW], f32, name="sb_log")
        sb_sig = consts.tile([128, NW], f32, name="sb_sig")

        nc.sync.dma_start(out=sb_ilo[:], in_=p_ilo[:])
        nc.sync.dma_start(out=sb_ihi[:], in_=p_ihi[:])
        nc.sync.dma_start(out=sb_dval[:], in_=p_dval[:])
        nc.sync.dma_start(out=sb_w[:], in_=p_w[:])
        nc.sync.dma_start(out=sb_bias[:], in_=p_bias[:])
        nc.sync.dma_start(out=sb_bcol[:], in_=p_bcol[:])
        nc.sync.dma_start(out=sb_rw[:], in_=p_rw[:])
        nc.sync.dma_start(out=sb_iota[:], in_=p_iota[:])

        def ag_half(k, half):
            """DMA h_sb half -> AG input bounce, then AllGather into hA/hB[k].

            The bounce keeps the SBUF layout ([128 part, HALF free]); the
            gather-row mapping row = rank*HALF + p*(HALF/128) + w accounts
            for it (see _prep_edges)."""
            w0 = 0 if half == 0 else NW // 2
            ag_in = (agA if half == 0 else agB)[k]
            ag_out = (hA if half == 0 else hB)[k]
            nc.sync.dma_start(
                out=ag_in[:, :], in_=sb_h[:, w0 * 128:(w0 + NW // 2) * 128])
            nc.gpsimd.collective_compute(
                "AllGather", OP.bypass, replica_groups=groups,
                ins=[ag_in[:, :]], outs=[ag_out[:, :]],
            )

        # ---- h0 = x @ emb_W + emb_b (sharded; xT comes in pre-transposed fp16)
        for t in range(NW):
            xt = xpool.tile([128, 128], f16, name="xt")
            nc.sync.dma_start(out=xt[:], in_=p_xT[:, t * 128:(t + 1) * 128])
            ps = head_ps.tile([128, 128], f32, name="h0ps")
            nc.tensor.matmul(ps[:], xt[:], sb_w[:, 0:128], start=True, stop=True)
            hs = sb_h[:, t * 128:(t + 1) * 128]
            nc.vector.scalar_tensor_tensor(
                out=hs, in0=ps[:], scalar=0.0, in1=sb_bias[:, 0:128],
                op0=OP.bypass, op1=OP.add)
            if t == NW // 2 - 1:
                ag_half(0, 0)
            if t == NW - 1:
                ag_half(0, 1)

        # ---- 4 aggregation passes
        for i in range(n_passes):
            for s in range(n_stages):
                w0, w1 = s * STAGE_W, min(s * STAGE_W + STAGE_W, NW)
                lo_c0, lo_c1 = LO_OFF[w0], LO_OFF[w1]
                hi_c0, hi_c1 = HI_OFF[w0], HI_OFF[w1]
                nlo, nhi = lo_c1 - lo_c0, hi_c1 - hi_c0
                # dma_gather crashes above ~1024 indices/call: split in
                # sub-calls of <= GCAP chunks.
                GCAP = 8
                ml = mlo_pool.tile([128, nlo, 128], f16, name="ml", tag="ml")
                for g0 in range(0, nlo, GCAP):
                    g1 = min(g0 + GCAP, nlo)
                    nc.gpsimd.dma_gather(
                        ml[:, g0:g1, :], hA[i][:, :],
                        sb_ilo[:, (lo_c0 + g0) * 8:(lo_c0 + g1) * 8],
                        (g1 - g0) * 128, (g1 - g0) * 128, 128, elem_step=128)
                mh = mhi_pool.tile([128, nhi, 128], f16, name="mh", tag="mh")
                for g0 in range(0, nhi, GCAP):
                    g1 = min(g0 + GCAP, nhi)
                    nc.gpsimd.dma_gather(
                        mh[:, g0:g1, :], hB[i][:, :],
                        sb_ihi[:, (hi_c0 + g0) * 8:(hi_c0 + g1) * 8],
                        (g1 - g0) * 128, (g1 - g0) * 128, 128, elem_step=128)

                for w in range(w0, w1):
                    ncl, nchh = NCH_LO[w], NCH_HI[w]
                    nct = ncl + nchh
                    c0 = CAT_OFF[w]
                    S = s_pool.tile([128, nct, 128], f16, name="S", tag="S")
                    dv3 = sb_dval[:, c0:c0 + nct].unsqueeze(2).broadcast_to(
                        (128, nct, 128))
                    io3 = sb_iota[:].unsqueeze(1).broadcast_to((128, nct, 128))
                    nc.vector.scalar_tensor_tensor(
                        out=S[:], in0=dv3, scalar=0.0, in1=io3,
                        op0=OP.bypass, op1=OP.is_equal)
                    ps = agg_ps.tile([128, 128], f32, name="aggps", tag="aggps")
                    for c in range(ncl):
                        nc.tensor.matmul(
                            ps[:], ml[:, (LO_OFF[w] - lo_c0) + c, :], S[:, c, :],
                            start=(c == 0), stop=False)
                    for c in range(nchh):
                        nc.tensor.matmul(
                            ps[:], mh[:, (HI_OFF[w] - hi_c0) + c, :], S[:, ncl + c, :],
                            start=False, stop=(c == nchh - 1))
                    at = agg_sb.tile([128, 128], f16, name="at", tag="at")
                    nc.any.tensor_copy(out=at[:], in_=ps[:])

                    if i < 3:
                        hp = head_ps.tile([128, 128], f32, name="hps", tag="hps")
                        nc.tensor.matmul(
                            hp[:], at[:], sb_w[:, (1 + i) * 128:(2 + i) * 128],
                            start=True, stop=True)
                        hs = sb_h[:, w * 128:(w + 1) * 128]
                        nc.vector.scalar_tensor_tensor(
                            out=hs, in0=hp[:], scalar=0.0,
                            in1=sb_bias[:, (1 + i) * 128:(2 + i) * 128],
                            op0=OP.bypass, op1=OP.add)
                        nc.scalar.activation(hs, hs, AF.Relu)
                    if i >= 1:
                        j = i - 1
                        sp = head_ps.tile([128, 128], f32, name="sps", tag="hps")
                        nc.tensor.matmul(
                            sp[:], sb_w[:, (4 + j) * 128:(5 + j) * 128], at[:],
                            start=True, stop=True)
                        st = sb_skipT[:, w * 128:(w + 1) * 128]
                        nc.scalar.activation(
                            st, sp[:], AF.Relu, bias=sb_bcol[:, j:j + 1])
                        lp = log_ps.tile([128, 1], f32, name="lp", tag="lp")
                        nc.tensor.matmul(
                            lp[:], st, sb_rw[:, j:j + 1], start=True, stop=True)
                        dl_ = sb_log[:, w:w + 1]
                        if j == 0:
                            nc.any.tensor_copy(out=dl_, in_=lp[:])
                        else:
                            nc.vector.scalar_tensor_tensor(
                                out=dl_, in0=lp[:], scalar=0.0, in1=dl_,
                                op0=OP.bypass, op1=OP.add)
                    if i < 3 and w == NW // 2 - 1:
                        ag_half(i + 1, 0)
                    if i < 3 and w == NW - 1:
                        ag_half(i + 1, 1)
            if i >= 1:
                j = i - 1
                nc.sync.dma_start(
                    out=o_feat[j * 128:(j + 1) * 128, :], in_=sb_skipT[:, :])

        if n_passes >= 2:
            nc.scalar.activation(sb_sig[:], sb_log[:], AF.Sigmoid)
            nc.sync.dma_start(out=o_log[:], in_=sb_log[:])
            nc.sync.dma_start(out=o_sig[:], in_=sb_sig[:])

    nc.compile()
    return nc


def _get_program(edge_index):
    n_passes = int(os.environ.get("GCN_PASSES", "4"))
    key = (hashlib.sha1(np.ascontiguousarray(edge_index).tobytes()).hexdigest(),
           n_passes)
    if key not in _cache:
        sched, ilo, ihi, dval = _prep_edges(edge_index)
        nc = _build_program(sched, n_passes)
        _cache[key] = (nc, sched, ilo, ihi, dval)
    return _cache[key]


def kernel(x, edge_index, emb_W, emb_b, gcn_W, gcn_b, skip_W, skip_b, readout_W):
    from concourse.bass_utils import run_bass_kernel_spmd

    x = np.asarray(x, np.float32)
    edge_index = np.asarray(edge_index)
    emb_W = np.asarray(emb_W, np.float32)
    emb_b = np.asarray(emb_b, np.float32)
    gcn_W = np.asarray(gcn_W, np.float32)
    gcn_b = np.asarray(gcn_b, np.float32)
    skip_W = np.asarray(skip_W, np.float32)
    skip_b = np.asarray(skip_b, np.float32)
    readout_W = np.asarray(readout_W, np.float32)

    nc, sched, ilo, ihi, dval = _get_program(edge_index)

    # constants shared by all cores
    wcat = np.concatenate(
        [emb_W, gcn_W[0], gcn_W[1], gcn_W[2], skip_W[0], skip_W[1], skip_W[2]],
        axis=1).astype(np.float16)
    bias_bc = np.concatenate(
        [np.tile(b[None, :], (128, 1)) for b in (emb_b, gcn_b[0], gcn_b[1], gcn_b[2])],
        axis=1).astype(np.float32)
    bias_col = np.stack(
        [skip_b[0], skip_b[1], skip_b[2], np.zeros(128, np.float32)], axis=1
    ).astype(np.float32)
    rw = np.stack(
        [readout_W[0:128, 0], readout_W[128:256, 0], readout_W[256:384, 0],
         np.zeros(128, np.float32)], axis=1).astype(np.float32)
    iota = np.tile(np.arange(128, dtype=np.float16)[None, :], (128, 1))

    xpad = np.zeros((NP_, D), np.float32)
    xpad[:N_RAW] = x

    in_maps = []
    for r in range(NCORES):
        xT = np.ascontiguousarray(
            xpad[r * SHARD:(r + 1) * SHARD].T).astype(np.float16)
        in_maps.append({
            "xT": xT,
            "idx_lo": ilo[r], "idx_hi": ihi[r], "dval": dval[r],
            "wcat": wcat, "bias_bc": bias_bc, "bias_col": bias_col,
            "rw": rw, "iota": iota,
        })

    trace = os.environ.get("GCN_TRACE", "") in ("1", "true")
    res = run_bass_kernel_spmd(
        nc, in_maps, list(range(NCORES)), trace=trace)
    kernel._last_result = res

    out_feat = np.concatenate(
        [res.results[r]["out_featT"].T for r in range(NCORES)], axis=0)[:N_RAW]
    logits = np.concatenate(
        [res.results[r]["logits"].T.reshape(SHARD) for r in range(NCORES)])[:N_RAW]
    sig = np.concatenate(
        [res.results[r]["sig"].T.reshape(SHARD) for r in range(NCORES)])[:N_RAW]
    return (out_feat.astype(np.float32), logits.astype(np.float32),
            sig.astype(np.float32))
